# revision 1
# baseline (speedup 1.0000x reference)
"""CenterLoss kernel for 8x Trainium2 NeuronCores.

Algorithm (matches the jax reference):
  counts[c] = #samples of class c; sums[c,:] = sum of embeddings of class c
  means = sums / max(counts,1)
  norms[i] = ||e_i - means[t_i]||
  loss = sum_c (1/n_c) * sum_{i in c} norms[i]   (empty classes contribute 0)
       = sum_i w_{t_i} * norms[i],  w_c = (n_c>0)/n_c   <- no 2nd segment-sum

Device strategy (data-parallel over samples, N/8 per core):
  Pass 1: per 128-sample tile, a one-hot matmul on the PE accumulates
    sumsT[d, c] in PSUM across all tiles.  The weights are the fp16 embedding
    tile with dim-0 shifted by +512, so PSUM row 0 = 512*n_c + s0_c which
    yields exact counts (|s0_global| << 256) AND recoverable dim-0 sums.
    The ScalarE also computes per-sample squared norms s_i (pre-shift).
  AllReduce the [128,1000] f32 partial sums across the 8 cores.
  Build a [1024,128]-f32-row table in DRAM per class: fp16 mean (256B),
    w^2 = (n>0)/n^2 at f32 slot 64, ||m||^2 at f32 slot 65.
  Pass 2: embeddings stay resident in SBUF as fp16 (cast during pass 1,
    un-shifted in between).  dma_gather pulls each sample's table row; one
    fused DVE tensor_tensor_reduce per tile computes -2*e.m into a column
    buffer.  Then norms^2 = s - 2d + ||m||^2, wnorm = sqrt(relu(norms^2*w^2))
    and the total are computed with a handful of fat [128, tiles] ops.
  Host sums the 8 per-core scalars.
"""

import sys

import numpy as np

for _p in ("/opt/trn_rl_repo", "/root/.axon_site/_ro/trn_rl_repo"):
    if _p not in sys.path:
        sys.path.append(_p)

D, C = 128, 1000
NCORES = 8
SHIFT = 512.0
TPC = 16  # tiles per E-staging chunk
GB = 8  # tiles per dma_gather call (1024 idx; 2048 overflows the SWDGE ring)

_cache = {}


def _build(n_loc, stage=3):
    import concourse.bacc as bacc
    import concourse.mybir as mybir
    import concourse.tile as tile
    from concourse import library_config

    f32 = mybir.dt.float32
    f16 = mybir.dt.float16
    i16 = mybir.dt.int16
    i32 = mybir.dt.int32
    AF = mybir.ActivationFunctionType
    ALU = mybir.AluOpType
    AX = mybir.AxisListType

    tiles = n_loc // 128
    chunks = tiles // TPC

    nc = bacc.Bacc(
        "TRN2",
        target_bir_lowering=False,
        debug=False,
        enable_asserts=False,
        num_devices=NCORES,
    )

    emb = nc.dram_tensor("emb", [n_loc, D], f32, kind="ExternalInput")
    tgtf = nc.dram_tensor("tgtf", [128, tiles], f32, kind="ExternalInput")
    gidx = nc.dram_tensor("gidx", [128, n_loc // 16], i16, kind="ExternalInput")
    iota = nc.dram_tensor("iota", [128, C], f16, kind="ExternalInput")
    ident = nc.dram_tensor("ident", [128, 128], f32, kind="ExternalInput")
    out = nc.dram_tensor("out", [1, 1], f32, kind="ExternalOutput")

    # one DMA drops TPC tiles into SBUF [128, TPC, 128]:
    # (p, j, d) <- emb[(chunk*TPC + j)*128 + p, d]
    emb_t = emb.ap().rearrange("(c j p) d -> c p j d", p=128, j=TPC)
    gcols = GB * 8  # gather-index columns per batch

    with tile.TileContext(nc) as tc:
        with (
            tc.tile_pool(name="const", bufs=1) as constp,
            tc.tile_pool(name="big", bufs=1) as bigp,
            tc.tile_pool(name="xfer", bufs=2) as xferp,
            tc.tile_pool(name="gat", bufs=2) as gatp,
            tc.tile_pool(name="oh", bufs=3) as ohp,
            tc.tile_pool(name="tmp1k", bufs=1) as tmp1kp,
            tc.tile_pool(name="small", bufs=2) as smallp,
            tc.tile_pool(name="acc1", bufs=1, space="PSUM") as psump,
            tc.tile_pool(name="ptr", bufs=2, space="PSUM") as psumtp,
            tc.tile_pool(name="dram", bufs=1, space="DRAM") as dramp,
        ):
            # GPSIMD library carrying the dma_gather Q7 kernel
            nc.gpsimd.load_library(library_config.mlp)

            # ---- constants ----
            iota_sb = constp.tile([128, C], f16)
            nc.sync.dma_start(iota_sb[:], iota.ap())
            tgt_sb = constp.tile([128, tiles], f32)
            nc.sync.dma_start(tgt_sb[:], tgtf.ap())
            ident_sb = constp.tile([128, 128], f32)
            nc.sync.dma_start(ident_sb[:], ident.ap())
            ones_col = constp.tile([128, 1], f32)
            nc.vector.memset(ones_col[:], 1.0)
            ones_row = constp.tile([1, 128], f32)
            nc.vector.memset(ones_row[:], 1.0)

            resident = bigp.tile([128, tiles * D], f16, tag="resident")
            res3 = resident[:].rearrange("p (j d) -> p j d", d=D)

            psum_acc = psump.tile([128, C], f32, tag="acc")

            # =================== PASS 1 ===================
            for cki in range(chunks):
                est = xferp.tile([128, TPC, D], f32, tag="xfer")
                nc.sync.dma_start(est[:], emb_t[cki])
                for j16 in range(TPC):
                    j = cki * TPC + j16
                    rt = resident[:, j * D : (j + 1) * D]
                    # cast f32 -> fp16 (resident copy; also the matmul weights)
                    nc.scalar.copy(rt, est[:, j16, :])
                for j16 in range(TPC):
                    j = cki * TPC + j16
                    rt = resident[:, j * D : (j + 1) * D]
                    # dim-0 +512 on the fp16 copy (counts channel)
                    nc.vector.tensor_scalar_add(rt[:, 0:1], rt[:, 0:1], SHIFT)
                    # one-hot [128 samples, 1000] fp16
                    oh = ohp.tile([128, C], f16, tag="oh")
                    nc.vector.tensor_scalar(
                        oh[:], iota_sb[:], tgt_sb[:, j : j + 1], None,
                        op0=ALU.is_equal,
                    )
                    first, last = j == 0, j == tiles - 1
                    nc.tensor.matmul(
                        psum_acc[:, 0:512], rt, oh[:, 0:512],
                        start=first, stop=last,
                    )
                    nc.tensor.matmul(
                        psum_acc[:, 512:C], rt, oh[:, 512:C],
                        start=first, stop=last,
                    )

            # =================== ALL-REDUCE ===================
            gsums = constp.tile([128, C], f32)
            nc.scalar.copy(gsums[:], psum_acc[:])
            ar_in = dramp.tile([128, C], f32)
            ar_out = dramp.tile([128, C], f32)
            nc.sync.dma_start(ar_in[:], gsums[:])
            nc.gpsimd.collective_compute(
                "AllReduce",
                ALU.add,
                replica_groups=[list(range(NCORES))],
                ins=[ar_in.opt()],
                outs=[ar_out.opt()],
            )
            nc.sync.dma_start(gsums[:], ar_out[:])

            if stage >= 2:
                # =================== TABLE BUILD ===================
                row0 = gsums[0:1, :]
                n_i32 = tmp1kp.tile([1, C], i32, tag="t1k_a")
                # n = round(row0/512): frac = s0/512 in (-0.25, 0.25), so
                # truncate(n + 0.25 + frac) == n under either rounding mode.
                nc.vector.tensor_scalar(
                    n_i32[:], row0, 1.0 / SHIFT, 0.25, op0=ALU.mult, op1=ALU.add
                )
                n_f = constp.tile([1, C], f32)
                nc.vector.tensor_copy(n_f[:], n_i32[:])
                # s0 = row0 - 512*n -> rewrite gsums row 0 with true dim-0 sums
                neg = tmp1kp.tile([1, C], f32, tag="t1k_b")
                nc.vector.tensor_scalar_mul(neg[:], n_f[:], -SHIFT)
                nc.vector.tensor_add(row0, gsums[0:1, :], neg[:])
                # inv = 1/max(n,1); w2 = inv*inv*(n>0)
                nmax = tmp1kp.tile([1, C], f32, tag="t1k_b")
                nc.vector.tensor_scalar_max(nmax[:], n_f[:], 1.0)
                inv = constp.tile([1, C], f32)
                nc.vector.reciprocal(inv[:], nmax[:])
                mask = tmp1kp.tile([1, C], f32, tag="t1k_b")
                nc.vector.tensor_scalar(mask[:], n_f[:], 0.5, None, op0=ALU.is_gt)
                w2 = constp.tile([1, C], f32)
                nc.vector.tensor_mul(w2[:], inv[:], inv[:])
                nc.vector.tensor_mul(w2[:], w2[:], mask[:])

                # broadcast inv across partitions via PE outer product
                pinv = psump.tile([128, C], f32, tag="pinv")
                nc.tensor.matmul(
                    pinv[:, 0:512], ones_row[:], inv[:, 0:512],
                    start=True, stop=True,
                )
                nc.tensor.matmul(
                    pinv[:, 512:C], ones_row[:], inv[:, 512:C],
                    start=True, stop=True,
                )
                meansT = constp.tile([128, C], f32)
                nc.vector.tensor_mul(meansT[:], gsums[:], pinv[:])
                # keep dim-0 shifted so it cancels against the resident shift
                nc.vector.tensor_scalar_add(meansT[0:1, :], meansT[0:1, :], SHIFT)

                # transpose to [class, d] rows; pack fp16 means + f32 w2
                rowbuf = constp.tile([128, 8, 128], f32)
                nc.vector.memset(rowbuf[:], 0.0)
                rowbuf16 = rowbuf[:].bitcast(f16)  # [128, 8, 256]
                for c8 in range(8):
                    cl = c8 * 128
                    ncl = min(128, C - cl)
                    tp = psumtp.tile([128, 128], f32, tag="tp")
                    nc.tensor.transpose(
                        tp[0:ncl, :], meansT[:, cl : cl + ncl], ident_sb[:]
                    )
                    nc.scalar.copy(rowbuf16[0:ncl, c8, 0:128], tp[0:ncl, :])
                    tpw = psumtp.tile([128, 1], f32, tag="tp")
                    nc.tensor.transpose(
                        tpw[0:ncl, :], w2[0:1, cl : cl + ncl],
                        ident_sb[0:1, 0:1],
                    )
                    nc.scalar.copy(rowbuf[0:ncl, c8, 64:65], tpw[0:ncl, :])

                table = nc.dram_tensor("table", [1024, 128], f32, kind="Internal")
                tbl_v = table.ap().rearrange("(c p) d -> p c d", p=128)
                nc.sync.dma_start(tbl_v, rowbuf[:])

            if stage >= 3:
                # =================== PASS 2 ===================
                # gathered row (fp16 view): [0:128]=mean, f32 slot 64=w2
                acc = constp.tile([128, 1], f32)
                nc.vector.memset(acc[:], 0.0)
                for bi in range(tiles // GB):
                    gslice = smallp.tile([128, gcols], i16, tag="gslice")
                    nc.sync.dma_start(
                        gslice[:], gidx.ap()[:, bi * gcols : (bi + 1) * gcols]
                    )
                    gt = gatp.tile([128, GB, 128], f32, tag="gt")
                    gt16 = gt[:].bitcast(f16)  # [128, GB, 256]
                    nc.gpsimd.dma_gather(
                        gt[:],
                        table.ap(),
                        gslice[:],
                        num_idxs=GB * 128,
                        num_idxs_reg=GB * 128,
                        elem_size=128,
                    )
                    for j16 in range(GB):
                        j = bi * GB + j16
                        diff = smallp.tile([128, D], f16, tag="diff")
                        nc.vector.tensor_sub(
                            diff[:], res3[:, j, :], gt16[:, j16, 0:128]
                        )
                        sq = smallp.tile([128, D], f16, tag="sq")
                        sqn = smallp.tile([128, 1], f32, tag="sqn")
                        nc.scalar.activation(
                            sq[:], diff[:], AF.Square, accum_out=sqn[:]
                        )
                        wn = smallp.tile([128, 1], f32, tag="wn")
                        nc.scalar.activation(
                            wn[:], sqn[:], AF.Sqrt, scale=gt[:, j16, 64:65]
                        )
                        nc.vector.tensor_add(acc[:], acc[:], wn[:])

                # =================== FINAL REDUCE ===================
                fin = psumtp.tile([1, 1], f32, tag="tp")
                nc.tensor.matmul(fin[:], acc[:], ones_col[:], start=True, stop=True)
                fin_sb = smallp.tile([1, 1], f32, tag="fin_sb")
                nc.scalar.copy(fin_sb[:], fin[:])
                nc.sync.dma_start(out.ap(), fin_sb[:])
            elif stage == 2:
                fs = smallp.tile([1, 1], f32, tag="fin_sb")
                nc.scalar.copy(fs[:], w2[0:1, 0:1])
                nc.sync.dma_start(out.ap(), fs[:])
            else:
                fs = smallp.tile([1, 1], f32, tag="fin_sb")
                nc.scalar.copy(fs[:], gsums[0:1, 0:1])
                nc.sync.dma_start(out.ap(), fs[:])

    nc.compile()
    return nc


def _host_inputs(embeddeds, target, n_loc):
    """Build the per-core input maps."""
    tiles = n_loc // 128
    iota_np = np.broadcast_to(
        np.arange(C, dtype=np.float16)[None, :], (128, C)
    ).copy()
    ident_np = np.eye(128, dtype=np.float32)
    in_maps = []
    for r in range(NCORES):
        e = np.ascontiguousarray(embeddeds[r * n_loc : (r + 1) * n_loc])
        t = target[r * n_loc : (r + 1) * n_loc]
        # [128, tiles]: tgtf[p, j] = t[128j + p]
        tgtf_np = np.ascontiguousarray(t.reshape(tiles, 128).T.astype(np.float32))
        # [128, n_loc/16]: gidx[p, k] = t[16k + p%16], replicated to 128 rows
        g = t.reshape(n_loc // 16, 16).T.astype(np.int16)  # [16, n/16]
        gidx_np = np.ascontiguousarray(np.tile(g, (8, 1)))
        in_maps.append(
            {
                "emb": e,
                "tgtf": tgtf_np,
                "gidx": gidx_np,
                "iota": iota_np,
                "ident": ident_np,
            }
        )
    return in_maps


def kernel(embeddeds, target, _trace=False, _stage=3):
    from concourse import bass_utils

    embeddeds = np.asarray(embeddeds, dtype=np.float32)
    target = np.asarray(target, dtype=np.int32)
    n = embeddeds.shape[0]
    n_loc = n // NCORES

    key = (n_loc, _stage)
    if key not in _cache:
        _cache[key] = _build(n_loc, stage=_stage)
    nc = _cache[key]

    in_maps = _host_inputs(embeddeds, target, n_loc)
    res = bass_utils.run_bass_kernel_spmd(
        nc, in_maps, core_ids=list(range(NCORES)), trace=_trace
    )
    total = np.float64(0.0)
    for r in res.results:
        total += np.float64(r["out"][0, 0])
    kernel.last_results = res
    return np.asarray(np.float32(total))

